# revision 1
# baseline (speedup 1.0000x reference)
"""Binarized linear block (y = relu(batchnorm(x @ sign(W).T))) on 8 TRN2 NeuronCores.

Strategy:
  - Data-parallel shard of the batch dim (16384 -> 2048 rows/core).
  - Weights binarized + transposed + tiled on host, replicated to all cores
    as fp16 (+-1 is exact in fp16).
  - x is cast to fp16 on host and pre-transposed so the contraction dim (IN)
    lies on SBUF partitions; the matmul computes y^T tiles [o x b] so the
    BN batch statistics are per-partition free-dim reductions.
  - Sync-BN via tiny per-group AllGathers of (mean, second moment) over DRAM
    bounce buffers, software-pipelined against the matmul stream: group g's
    collective launches after group g+1's first channel tile, and its finish
    phase (gather + scale/shift + fused normalize+ReLU on ScalarE) is emitted
    two collectives later, so neither ScalarE's nor TensorE's strict-FIFO
    queues ever head-of-line block on collective latency. Stats are PE-
    transposed to [2g, 128] so bounce DMAs are a few 512B descriptors
    instead of 128 tiny ones.
  - Output is written as y^T tiles [m, 128, b_loc]; host transposes back.
"""

import numpy as np

_BN_EPS = 1e-5

_CACHE = {}


def _env(name):
    import os

    return bool(os.environ.get(name))


def _group_sizes(mt):
    # Pipelined sync-BN in small uniform groups. Each group's collective is
    # triggered right after its matmuls; its finish phase (gather, scale/
    # shift, normalize+store) is EMITTED one group later, so the collective
    # has a full group's compute time to land before ScalarE's strict-FIFO
    # queue reaches the finish instructions (otherwise they head-of-line
    # block the PSUM-draining Identity copies and stall the PE).
    if mt <= 2:
        return [mt]
    if mt <= 4:
        return [mt - 2, 1, 1]
    rest = mt - 2
    return [2] * (rest // 2) + ([1] if rest % 2 else []) + [1, 1]


def _build(n_cores, b_loc, in_dim, out_dim, b_total):
    import concourse.bass as bass  # noqa: F401
    import concourse.mybir as mybir
    import concourse.tile as tile
    from concourse import bacc

    f16 = mybir.dt.bfloat16 if _env("KBN_BF16") else mybir.dt.float16
    f32 = mybir.dt.float32
    AF = mybir.ActivationFunctionType
    ALU = mybir.AluOpType

    KT = in_dim // 128   # k tiles (contraction)
    MT = out_dim // 128  # output-channel tiles
    CH = min(512, b_loc)  # moving-operand chunk
    NCH = b_loc // CH    # batch chunks
    groups = _group_sizes(MT)

    nc = bacc.Bacc(
        "TRN2",
        target_bir_lowering=False,
        debug=False,
        enable_asserts=False,
        num_devices=n_cores,
    )

    # xt[p, n, k, b] so each batch-chunk DMA is contiguous per partition
    xt = nc.dram_tensor("xt", [128, NCH, KT, CH], f16, kind="ExternalInput")
    wt = nc.dram_tensor("wt", [MT, 128, KT, 128], f16, kind="ExternalInput")
    gmt = nc.dram_tensor("gmt", [128, MT], f32, kind="ExternalInput")
    bta = nc.dram_tensor("bta", [128, MT], f32, kind="ExternalInput")
    out = nc.dram_tensor("out", [MT, 128, b_loc], f32, kind="ExternalOutput")

    with tile.TileContext(nc) as tc:
        with (
            tc.tile_pool(name="xpool", bufs=1) as xpool,
            tc.tile_pool(name="wpool", bufs=4) as wpool,
            tc.tile_pool(name="ypool", bufs=MT) as ypool,
            tc.tile_pool(name="opool", bufs=4) as opool,
            tc.tile_pool(name="stat", bufs=1) as stat,
            tc.tile_pool(name="gstat", bufs=4) as gstat,
            tc.tile_pool(name="psum", bufs=4, space="PSUM") as psum,
            tc.tile_pool(name="psum2", bufs=2, space="PSUM") as psum2,
            tc.tile_pool(name="dram", bufs=4, space="DRAM") as dram,
        ):
            xt_sb = xpool.tile([128, NCH, KT, CH], f16)
            # first weight tile + first x chunk lead; the rest of x streams in
            wts = []
            for m in range(min(2, MT)):
                wt_m = wpool.tile([128, KT, 128], f16, tag="wt")
                nc.sync.dma_start(wt_m[:], wt.ap()[m])
                wts.append(wt_m)
            # chunk 0 in k-halves so the first matmuls can start sooner
            kh = max(KT // 2, 1)
            nc.sync.dma_start(xt_sb[:, 0, :kh], xt.ap()[:, 0, :kh])
            nc.sync.dma_start(xt_sb[:, 0, kh:], xt.ap()[:, 0, kh:])
            for n in range(1, NCH):
                nc.sync.dma_start(xt_sb[:, n], xt.ap()[:, n])

            gamma_sb = stat.tile([128, MT], f32)
            beta_sb = stat.tile([128, MT], f32)
            nc.gpsimd.dma_start(gamma_sb[:], gmt.ap())
            nc.gpsimd.dma_start(beta_sb[:], bta.ap())

            eps_t = stat.tile([128, 1], f32)
            nc.vector.memset(eps_t[:], _BN_EPS)

            # identity for PE-based transposes of the tiny stats tensors:
            # a [128, c] SBUF->DRAM DMA is 128 c*4B descriptors (~20us even
            # on HWDGE), while the [c, 128] transposed layout is c 512B
            # descriptors. The two transposes cost ~300ns of PE each.
            from concourse.masks import make_identity

            ident = stat.tile([128, 128], f32)
            make_identity(nc, ident[:])

            yts = [None] * MT
            last_mm = [None]  # most recently emitted matmul instruction

            def emit_chunk(m, wt_m, bns, j, n):
                """One (channel-tile, batch-chunk): 16 matmuls + epilogues."""
                ns = slice(n * CH, (n + 1) * CH)
                ps = psum.tile([128, CH], f32)
                for k in range(KT):
                    last_mm[0] = nc.tensor.matmul(
                        ps[:],
                        wt_m[:, k, :],
                        xt_sb[:, n, k, :],
                        start=(k == 0),
                        stop=(k == KT - 1),
                    )
                # ScalarE: fp16 copy of y^T; VectorE: batch stats
                nc.scalar.activation(yts[m][:, ns], ps[:], AF.Identity)
                nc.vector.bn_stats(out=bns[:, j, n, :], in_=ps[:])

            def emit_collective(m0, gm, bns):
                """Pack the group's stats and launch its AllGather.

                Emitted after the NEXT group's first m-tile so the PE-queue
                transpose never waits on the DVE stats chain.
                """
                # local (mean, var) per channel tile in the group
                mv = gstat.tile([128, gm, 2], f32, tag="mv")
                for j in range(gm):
                    nc.vector.bn_aggr(out=mv[:, j, :], in_=bns[:, j])

                # cross-core summands packed as st = [a | b] with
                # a = mean/W, b = (var + mean^2)/W, then PE-transposed to
                # [2*gm, 128] so the bounce DMA is 2*gm big descriptors
                st = gstat.tile([128, 2 * gm], f32, tag="st")
                nc.vector.tensor_mul(st[:, gm:], mv[:, :, 0], mv[:, :, 0])
                nc.vector.tensor_add(st[:, gm:], mv[:, :, 1], st[:, gm:])
                nc.vector.tensor_scalar_mul(st[:, gm:], st[:, gm:], 1.0 / n_cores)
                nc.vector.tensor_scalar_mul(st[:, :gm], mv[:, :, 0], 1.0 / n_cores)

                psT = psum2.tile([2 * gm, 128], f32, tag="psT")
                nc.tensor.transpose(psT[:], st[:], ident[:])
                stT = gstat.tile([2 * gm, 128], f32, tag="stT")
                nc.vector.tensor_copy(stT[:], psT[:])

                bounce_out = None
                if n_cores > 1:
                    bounce_in = dram.tile([2 * gm, 128], f32, tag="bin")
                    nc.scalar.dma_start(bounce_in[:], stT[:])
                    # AllGather + local reduce: lower latency than an
                    # AllReduce for latency-dominated tiny messages
                    bounce_out = dram.tile(
                        [n_cores, 2 * gm, 128], f32, tag="bout"
                    )
                    nc.gpsimd.collective_compute(
                        "AllGather",
                        ALU.bypass,
                        replica_groups=[list(range(n_cores))],
                        ins=[bounce_in.opt()],
                        outs=[bounce_out.opt()],
                    )
                return (m0, gm, bounce_out, stT)

            def emit_finish(state):
                m0, gm, bounce_out, stT = state
                gstats = gstat.tile([128, 2 * gm], f32, tag="gstats")
                if n_cores > 1:
                    allT = gstat.tile([2 * gm, n_cores, 128], f32, tag="allT")
                    nc.scalar.dma_start(
                        allT[:], bounce_out[:].rearrange("r c f -> c r f")
                    )
                    w = n_cores
                    while w > 1:
                        w //= 2
                        nc.vector.tensor_add(
                            allT[:, :w], allT[:, :w], allT[:, w : 2 * w]
                        )
                    redT = allT[:, 0, :]
                else:
                    redT = stT[:]
                psB = psum2.tile([128, 2 * gm], f32, tag="psB")
                back_t = nc.tensor.transpose(
                    psB[:], redT, ident[: 2 * gm, : 2 * gm]
                )
                # pin behind the newest matmul so this collective-dependent
                # PE op can't head-of-line block the matmul stream
                if last_mm[0] is not None:
                    tile.add_dep_helper(
                        back_t.ins,
                        last_mm[0].ins,
                        sync=False,
                        reason="stats back-transpose after matmul stream",
                    )
                nc.vector.tensor_copy(gstats[:], psB[:])

                mean_t = gstats[:, :gm]  # global mean
                var_t = gstat.tile([128, gm], f32, tag="var")
                sd_t = gstat.tile([128, gm], f32, tag="sd")
                inv_t = gstat.tile([128, gm], f32, tag="inv")
                scale_t = gstat.tile([128, gm], f32, tag="scale")
                tmp_t = gstat.tile([128, gm], f32, tag="tmp")
                shift_t = gstat.tile([128, gm], f32, tag="shift")
                nc.vector.tensor_mul(var_t[:], mean_t, mean_t)
                nc.vector.tensor_sub(var_t[:], gstats[:, gm:], var_t[:])
                nc.scalar.activation(sd_t[:], var_t[:], AF.Sqrt, bias=eps_t[:])
                nc.vector.reciprocal(inv_t[:], sd_t[:])
                nc.vector.tensor_mul(
                    scale_t[:], gamma_sb[:, m0 : m0 + gm], inv_t[:]
                )
                nc.vector.tensor_mul(tmp_t[:], mean_t, scale_t[:])
                nc.vector.tensor_sub(
                    shift_t[:], beta_sb[:, m0 : m0 + gm], tmp_t[:]
                )

                for j, m in enumerate(range(m0, m0 + gm)):
                    out_m = opool.tile([128, b_loc], f32)
                    nc.scalar.activation(
                        out_m[:],
                        yts[m][:],
                        AF.Relu,
                        bias=shift_t[:, j : j + 1],
                        scale=scale_t[:, j : j + 1],
                    )
                    nc.sync.dma_start(out.ap()[m], out_m[:])

            # Pipeline: batch-chunk-outer within each group (halves the
            # cold-start HBM demand so the PE doesn't outrun the x DMAs).
            # Group g's collective block is emitted after group g+1's first
            # chunk pass (stats chain hides under those matmuls); its finish
            # is emitted two collectives later so even a slow AllGather
            # (CC-stream warmup ~60us on the first op) lands before the
            # engines reach the finish instructions. The last two groups are
            # single-tile with immediately-emitted collectives, spaced a
            # whole m-tile apart so they don't queue on the serial CC
            # stream: the final exposed AllGather is the 1-tile one.
            states = []
            pend_coll = None
            m0 = 0
            G = len(groups)
            for g, gm in enumerate(groups):
                ms = list(range(m0, m0 + gm))
                bns = gstat.tile([128, gm, NCH, 6], f32, tag="bns")
                wtiles = []
                for m in ms:
                    if m < len(wts):
                        wtiles.append(wts[m])
                    else:
                        wt_m = wpool.tile([128, KT, 128], f16, tag="wt")
                        nc.sync.dma_start(wt_m[:], wt.ap()[m])
                        wtiles.append(wt_m)
                    yt_m = ypool.tile([128, b_loc], f16, tag="yt")
                    yts[m] = yt_m
                for n in range(NCH):
                    for j, m in enumerate(ms):
                        emit_chunk(m, wtiles[j], bns, j, n)
                    if n == 0 and pend_coll is not None:
                        states.append(emit_collective(*pend_coll))
                        pend_coll = None
                        if len(states) > 3:
                            emit_finish(states.pop(0))
                if g >= G - 2:
                    states.append(emit_collective(m0, gm, bns))
                    if len(states) > 3:
                        emit_finish(states.pop(0))
                else:
                    pend_coll = (m0, gm, bns)
                m0 += gm
            if pend_coll is not None:
                states.append(emit_collective(*pend_coll))
            for state in states:
                emit_finish(state)

    nc.compile()
    return nc


def _get_nc(key):
    if key not in _CACHE:
        _CACHE[key] = _build(*key)
    return _CACHE[key]


def _prepare_in_maps(x, weight, gamma, beta, n_cores):
    b_total, in_dim = x.shape
    out_dim = weight.shape[0]
    b_loc = b_total // n_cores
    KT = in_dim // 128
    MT = out_dim // 128
    CH = min(512, b_loc)
    NCH = b_loc // CH

    # host-side marshalling (binarize / transpose / cast / tile)
    wb = np.where(weight >= 0, np.float32(1.0), np.float32(-1.0))
    # wt[m, p, k, o] = sign(W)[m*128+o, k*128+p]
    import ml_dtypes as _mld

    _hdt = _mld.bfloat16 if _env("KBN_BF16") else np.float16
    wt = np.ascontiguousarray(
        wb.reshape(MT, 128, KT, 128).transpose(0, 3, 2, 1).astype(_hdt)
    )
    gmt = np.ascontiguousarray(gamma.reshape(MT, 128).T.astype(np.float32))
    bta = np.ascontiguousarray(beta.reshape(MT, 128).T.astype(np.float32))

    import ml_dtypes

    x16 = x.astype(ml_dtypes.bfloat16 if _env("KBN_BF16") else np.float16)
    in_maps = []
    for c in range(n_cores):
        xc = x16[c * b_loc : (c + 1) * b_loc]  # [b, in]
        # xt[p, n, k, b] = x[b0 + n*CH + b, k*128+p]
        xt = np.ascontiguousarray(
            xc.reshape(NCH, CH, KT, 128).transpose(3, 0, 2, 1)
        )
        in_maps.append({"xt": xt, "wt": wt, "gmt": gmt, "bta": bta})
    return in_maps


def _gather_out(results, b_total, out_dim, n_cores):
    b_loc = b_total // n_cores
    out = np.empty((b_total, out_dim), dtype=np.float32)
    for c in range(n_cores):
        oc = np.asarray(results[c]["out"]).reshape(out_dim // 128, 128, b_loc)
        out[c * b_loc : (c + 1) * b_loc] = oc.transpose(2, 0, 1).reshape(
            b_loc, out_dim
        )
    return out


def kernel(x, weight, gamma, beta):
    from concourse.bass_utils import run_bass_kernel_spmd

    n_cores = 8
    b_total, in_dim = x.shape
    out_dim = weight.shape[0]

    nc = _get_nc((n_cores, b_total // n_cores, in_dim, out_dim, b_total))
    in_maps = _prepare_in_maps(x, weight, gamma, beta, n_cores)
    res = run_bass_kernel_spmd(nc, in_maps, list(range(n_cores)))
    return _gather_out(res.results, b_total, out_dim, n_cores)

